# revision 11
# baseline (speedup 1.0000x reference)
"""Fastfood layer kernel for 8x Trainium2 NeuronCores.

Reference computation per row r (d=1024, m=8 blocks):
    v_j = S_j * H( G_j * gather_{P_j}( H( B_j * x_r ) ) ) / sqrt(d)
    out[r, j*d+k] = cos(v_j[k] + 2*pi*u[j*d+k]) * sqrt(2/8192)

Since B/G/S/P are row-independent, the linear part collapses to a fixed
matrix W (1024 x 8192): out = cos(x @ W + phase) * kscale.  W is built on
host with an exact float64 FWHT; the device does a tiled fp32r matmul with
a range-reduced sin epilogue, data-parallel over the 8192 rows (1024/core).

Epilogue per PSUM tile (theta = x@W + 2*pi*u + pi/2, in radians):
    k   = round_to_i32(theta / 2pi)      (DVE dtype-convert rounds to nearest)
    y   = theta - 2pi*k                  in [-pi, pi]
    out = kscale * sin(y)                (ACT Sin, ~5e-6 abs err on [-pi,pi])
"""

import math

import numpy as np

import concourse.bass as bass
import concourse.mybir as mybir
import concourse.tile as tile
from concourse import bacc
from concourse.bass_utils import run_bass_kernel_spmd

D = 1024
M_BLOCKS = 8
OUT_DIM = 8192
N_CORES = 8
ROWS_PER_CORE = 1024

# float32r streams fp32 through the PE at 1 cycle/row (vs 4 for float32).
MM_DT = mybir.dt.float32r

R_TILE = 128   # output rows per PSUM tile (partitions)
C_TILE = 512   # output cols per PSUM tile (one fp32 PSUM bank)
K_TILE = 128   # contraction chunk (partitions)

TWO_PI = 2.0 * math.pi


def _fwht_axis0(a: np.ndarray) -> np.ndarray:
    """Unnormalized FWHT along axis 0, matching the reference's
    recursive-cat (Sylvester/natural) ordering."""
    n = a.shape[0]
    h = 1
    while h < n:
        a = a.reshape(n // (2 * h), 2, h, *a.shape[1:])
        x = a[:, 0] + a[:, 1]
        y = a[:, 0] - a[:, 1]
        a = np.stack([x, y], axis=1).reshape(n, *a.shape[3:])
        h *= 2
    return a


def _build_w_and_phase(B, G, S, P, u_rand):
    """Host-side exact precompute of the fused weight matrix and phase row."""
    Hmat = _fwht_axis0(np.eye(D, dtype=np.float64))
    norm = 1.0 / math.sqrt(D)
    W = np.empty((D, OUT_DIM), dtype=np.float64)
    for j in range(M_BLOCKS):
        # out_j = (1/sqrt(d)) * D_S H D_G Gamma_j H D_B x_r
        A = Hmat * B[j].astype(np.float64)[None, :]       # H D_B
        A = A[P[j].astype(np.int64), :]                   # row gather by P_j
        A = A * G[j].astype(np.float64)[:, None]          # D_G
        A = _fwht_axis0(A)                                # H @ (.)
        A = A * S[j].astype(np.float64)[:, None] * norm   # D_S / sqrt(d)
        W[:, j * D:(j + 1) * D] = A.T
    phase = (TWO_PI * u_rand.astype(np.float64) + 0.5 * math.pi)
    return W.astype(np.float32), phase.astype(np.float32)


def _build_nc():
    nc = bacc.Bacc("TRN2", target_bir_lowering=False, debug=False)
    xT_ext = nc.declare_dram_parameter("xT", [D, ROWS_PER_CORE], MM_DT,
                                       isOutput=False)
    w_ext = nc.declare_dram_parameter("W", [D, OUT_DIM], MM_DT,
                                      isOutput=False)
    ph_ext = nc.declare_dram_parameter("phase_bcast", [R_TILE, OUT_DIM],
                                       mybir.dt.float32, isOutput=False)
    out_ext = nc.declare_dram_parameter("out", [ROWS_PER_CORE, OUT_DIM],
                                        mybir.dt.float32, isOutput=True)

    n_r = ROWS_PER_CORE // R_TILE   # 8
    n_c = OUT_DIM // C_TILE         # 16
    n_k = D // K_TILE               # 8
    kscale = math.sqrt(2.0 / OUT_DIM)

    with tile.TileContext(nc) as tc:
        with (
            tc.tile_pool(name="xt", bufs=1) as xt_pool,
            tc.tile_pool(name="ph", bufs=1) as ph_pool,
            tc.tile_pool(name="w", bufs=16) as w_pool,
            tc.tile_pool(name="ep", bufs=3) as ep_pool,
            tc.tile_pool(name="ob", bufs=4) as out_pool,
            tc.tile_pool(name="ps", bufs=4, space="PSUM") as psum_pool,
        ):
            # Resident activations: xT as n_k tiles of [128, ROWS_PER_CORE]
            xt_tiles = []
            for k in range(n_k):
                t = xt_pool.tile([K_TILE, ROWS_PER_CORE], MM_DT, tag=f"xt{k}")
                nc.sync.dma_start(t[:], xT_ext[k * K_TILE:(k + 1) * K_TILE, :])
                xt_tiles.append(t)
            # Resident phase broadcast tile [128, OUT_DIM] (radians, +pi/2)
            ph_tile = ph_pool.tile([R_TILE, OUT_DIM], mybir.dt.float32, tag="ph")
            nc.sync.dma_start(ph_tile[:], ph_ext[:, :])
            # Per-partition zero bias for ACT Sin
            bias_tile = ph_pool.tile([R_TILE, 1], mybir.dt.float32, tag="bias")
            nc.vector.memset(bias_tile[:], 0.0)

            for c in range(n_c):
                w_tiles = []
                for k in range(n_k):
                    wt = w_pool.tile([K_TILE, C_TILE], MM_DT, tag="w")
                    nc.sync.dma_start(
                        wt[:], w_ext[k * K_TILE:(k + 1) * K_TILE,
                                     c * C_TILE:(c + 1) * C_TILE])
                    w_tiles.append(wt)
                for r in range(n_r):
                    ps = psum_pool.tile([R_TILE, C_TILE], mybir.dt.float32)
                    for k in range(n_k):
                        nc.tensor.matmul(
                            ps[:],
                            xt_tiles[k][:, r * R_TILE:(r + 1) * R_TILE],
                            w_tiles[k][:],
                            start=(k == 0), stop=(k == n_k - 1))
                    # theta = psum + phase
                    th = ep_pool.tile([R_TILE, C_TILE], mybir.dt.float32, tag="t")
                    nc.vector.scalar_tensor_tensor(
                        out=th[:], in0=ps[:], scalar=1.0,
                        in1=ph_tile[:, c * C_TILE:(c + 1) * C_TILE],
                        op0=mybir.AluOpType.mult, op1=mybir.AluOpType.add)
                    # k = round(theta / 2pi)  (f32 -> i32 convert rounds)
                    ki = ep_pool.tile([R_TILE, C_TILE], mybir.dt.int32, tag="k")
                    nc.vector.tensor_scalar(
                        out=ki[:], in0=th[:], scalar1=1.0 / TWO_PI,
                        scalar2=None, op0=mybir.AluOpType.mult)
                    # y = theta - 2pi*k  in [-pi, pi]
                    y = ep_pool.tile([R_TILE, C_TILE], mybir.dt.float32, tag="y")
                    nc.vector.scalar_tensor_tensor(
                        out=y[:], in0=ki[:], scalar=-TWO_PI, in1=th[:],
                        op0=mybir.AluOpType.mult, op1=mybir.AluOpType.add)
                    # s = sin(y)
                    s = ep_pool.tile([R_TILE, C_TILE], mybir.dt.float32, tag="s")
                    nc.scalar.activation(s[:], y[:],
                                         mybir.ActivationFunctionType.Sin,
                                         bias=bias_tile[:, 0:1], scale=1.0)
                    ob = out_pool.tile([R_TILE, C_TILE], mybir.dt.float32, tag="o")
                    nc.vector.tensor_scalar_mul(ob[:], s[:], kscale)
                    nc.sync.dma_start(
                        out_ext[r * R_TILE:(r + 1) * R_TILE,
                                c * C_TILE:(c + 1) * C_TILE], ob[:])
    nc.compile()
    return nc


_NC_CACHE = None


def kernel(x, B, G, S, P, u_rand):
    global _NC_CACHE
    W_dev, phase = _build_w_and_phase(B, G, S, P, u_rand)
    phase_bcast = np.broadcast_to(phase[None, :], (R_TILE, OUT_DIM)).copy()

    if _NC_CACHE is None:
        _NC_CACHE = _build_nc()
    nc = _NC_CACHE

    x = np.ascontiguousarray(x, dtype=np.float32).reshape(N_CORES, ROWS_PER_CORE, D)
    in_maps = []
    for c in range(N_CORES):
        xT = np.ascontiguousarray(x[c].T)
        in_maps.append({"xT": xT, "W": W_dev, "phase_bcast": phase_bcast})

    global _LAST_IN_MAPS
    _LAST_IN_MAPS = in_maps
    res = run_bass_kernel_spmd(nc, in_maps, list(range(N_CORES))).results
    return np.concatenate([res[c]["out"] for c in range(N_CORES)], axis=0)


_LAST_IN_MAPS = None
